# revision 1
# baseline (speedup 1.0000x reference)
"""Multi-head attention (RoPE, causal) Bass kernel for 8 TRN2 NeuronCores.

Problem: x[2,2048,1024], 16 heads x 64 dim, causal mask, RoPE, f32.

Sharding: batch x head-group. Core c handles batch c//4 and the 4 heads
[4*(c%4), 4*(c%4)+4). Each core computes q/k/v projections for its head
slice, RoPE, causal attention, and a partial output projection against its
rows of Wo.T. The host sums the 4 partials per batch (the "all-reduce" of
the row-split output projection is done on the host during unsharding).

Device layout notes:
- x is passed pre-transposed per batch: xT [1024, 2048] so it can stream as
  the matmul moving operand.
- Wq/Wk rows are permuted per head to [e0..e15, o0..o15, e16..e31, o16..o31]
  (e/o = even/odd RoPE pair lanes) so the RoPE rotate-half becomes a
  16<->16 swap inside each 32-partition group (one DVE stream_shuffle).
- qT/kT/probs/v/attn-out/Wo run in bf16 (f32 PSUM accumulation); the QKV
  projection runs in float32r (full-rate one-pass fp32 matmul).
- Causal masking is done on the TensorEngine by accumulating a constant
  lower-triangular -1e30 tile (via matmul against identity) onto the
  diagonal score blocks; off-diagonal invalid blocks are never computed.
- Softmax denominators come free from the PV matmul by appending a ones
  column to v (lhsT = [v | 1] -> row 64 of the PV psum is sum(probs)).
"""

import numpy as np
import ml_dtypes

import concourse.bass as bass
import concourse.mybir as mybir
import concourse.tile as tile
from concourse import bacc
from concourse.bass_utils import run_bass_kernel_spmd

F32 = mybir.dt.float32
F32R = mybir.dt.float32r
BF16 = mybir.dt.bfloat16

B, S, D = 2, 2048, 1024
H, HD = 16, 64
NCORES = 8
HPC = 4          # heads per core
DQ = HPC * HD    # 256 projected dims per core
THETA = 10000.0
NEG = -1e30

_cached = {}
SIM_SAFE = False  # emit CoreSim-only zero-fill matmuls


def _r32(ap):
    return ap.bitcast(F32R)


def build_nc():
    """Build the single-core Bass graph (same NEFF runs SPMD on all 8)."""
    nc = bacc.Bacc("TRN2", target_bir_lowering=False)

    xt_d = nc.dram_tensor("xt", [D, S], F32R, kind="ExternalInput")
    wq_d = nc.dram_tensor("wq", [D, DQ], F32R, kind="ExternalInput")
    wk_d = nc.dram_tensor("wk", [D, DQ], F32R, kind="ExternalInput")
    wv_d = nc.dram_tensor("wv", [D, DQ], F32R, kind="ExternalInput")
    wo_d = nc.dram_tensor("wo", [DQ, D], BF16, kind="ExternalInput")
    cos_d = nc.dram_tensor("cos", [128, S], BF16, kind="ExternalInput")
    sin_d = nc.dram_tensor("sin", [128, S], BF16, kind="ExternalInput")
    tri_d = nc.dram_tensor("tri", [128, 128], BF16, kind="ExternalInput")
    ident_d = nc.dram_tensor("ident", [128, 128], BF16, kind="ExternalInput")
    out_d = nc.dram_tensor("out", [S, D], F32, kind="ExternalOutput")

    Exp = mybir.ActivationFunctionType.Exp
    SHUF = [(i + 16) % 32 for i in range(32)]  # 16<->16 swap per 32-group

    with tile.TileContext(nc) as tc:
        with (
            tc.tile_pool(name="consts", bufs=1) as consts,
            tc.tile_pool(name="big", bufs=8) as bigp,
            tc.tile_pool(name="qk", bufs=1) as qkp,
            tc.tile_pool(name="vsb", bufs=1) as vp,
            tc.tile_pool(name="rope", bufs=3) as ropep,
            tc.tile_pool(name="probs", bufs=4) as probsp,
            tc.tile_pool(name="small", bufs=3) as smallp,
            tc.tile_pool(name="ps", bufs=2, space="PSUM") as psp,
            tc.tile_pool(name="pv", bufs=4, space="PSUM") as pvp,
        ):
            # ---- constants (wq/first-x DMAs are emitted first below) ----
            wq_sb = consts.tile([128, 8, DQ], F32R, tag="wq")
            wk_sb = consts.tile([128, 8, DQ], F32R, tag="wk")
            wv_sb = consts.tile([128, 8, DQ], F32R, tag="wv")
            wo_sb = consts.tile([128, 2, D], BF16, tag="wo")
            nc.sync.dma_start(out=wo_sb, in_=wo_d.rearrange("(k p) m -> p k m", p=128))
            cs = {}
            for name, dram in (("cos", cos_d), ("sin", sin_d)):
                t = consts.tile([128, S], BF16, tag=name, name=name)
                nc.sync.dma_start(out=t, in_=dram[:, :])
                cs[name] = t
            tri_sb = consts.tile([128, 128], BF16, tag="tri")
            nc.sync.dma_start(out=tri_sb, in_=tri_d[:, :])
            ident_sb = consts.tile([128, 128], BF16, tag="ident")
            nc.sync.dma_start(out=ident_sb, in_=ident_d[:, :])
            zeros_sb = consts.tile([128, 384], BF16, tag="zeros")
            nc.vector.memset(zeros_sb, 0.0)

            # ---- xT resident tiles; first block + wq DMA'd ahead ----
            xt = []
            for k in range(8):
                t = bigp.tile([128, S], F32R, tag="big", name=f"xt{k}")
                xt.append(t)
            for k in range(8):
                nc.sync.dma_start(out=xt[k][:, 0:1024],
                                  in_=xt_d[128 * k:128 * (k + 1), 0:1024])
            for k in range(8):
                nc.sync.dma_start(
                    out=wq_sb[:, k, :],
                    in_=wq_d[128 * k:128 * (k + 1), :])
            for k in range(8):
                nc.sync.dma_start(
                    out=wk_sb[:, k, :],
                    in_=wk_d[128 * k:128 * (k + 1), :])
            for k in range(8):
                nc.sync.dma_start(
                    out=wv_sb[:, k, :],
                    in_=wv_d[128 * k:128 * (k + 1), :])

            # q/k destination tiles: [pair][128 rows = 2 heads x 64, S]
            qt = [qkp.tile([128, S], BF16, tag=f"qt{p}", name=f"qt{p}") for p in range(2)]
            kt = [qkp.tile([128, S], BF16, tag=f"kt{p}", name=f"kt{p}") for p in range(2)]
            # v tiles: per s-chunk [128, 4*65] ([v_h | 1] per head)
            vsb = [vp.tile([128, 4 * 65], BF16, tag=f"v{i}", name=f"v{i}") for i in range(16)]
            # attention output (pre out-proj): [pair][128 = 2 heads x 64 dv, S]
            ot = [qkp.tile([128, S], BF16, tag=f"ot{p}", name=f"ot{p}") for p in range(2)]

            # ---- phase 1: projections (+RoPE for q/k), streamed by s-block ----
            for n in range(2):
                ncol = slice(1024 * n, 1024 * (n + 1))
                if n > 0:
                    for k in range(8):
                        nc.sync.dma_start(
                            out=xt[k][:, ncol],
                            in_=xt_d[128 * k:128 * (k + 1), ncol],
                        )
                for w_sb, dst, cosn, sinn in (
                    (wq_sb, qt, "cos", "sin"),
                    (wk_sb, kt, "cos", "sin"),
                ):
                    for m in range(2):
                        ps = psp.tile([128, 2, 512], F32, tag="ps")
                        mcol = slice(128 * m, 128 * (m + 1))
                        for half in range(2):
                            n2 = slice(1024 * n + 512 * half,
                                       1024 * n + 512 * (half + 1))
                            for k in range(8):
                                nc.tensor.matmul(
                                    ps[:, half, :],
                                    lhsT=w_sb[:, k, mcol],
                                    rhs=xt[k][:, n2],
                                    start=(k == 0),
                                    stop=(k == 7),
                                )
                        # RoPE over both halves: dst = raw*cos + shuf(raw)*sin
                        wcol = slice(1024 * n, 1024 * (n + 1))
                        psf = ps.rearrange("p a b -> p (a b)")
                        raw = ropep.tile([128, 1024], BF16, tag="raw")
                        nc.vector.tensor_copy(raw, psf)
                        rot = ropep.tile([128, 1024], BF16, tag="rot")
                        nc.vector.stream_shuffle(rot, raw, SHUF)
                        t1 = ropep.tile([128, 1024], BF16, tag="rot", name="t1")
                        nc.vector.tensor_mul(t1, raw, cs[cosn][:, wcol])
                        t2 = ropep.tile([128, 1024], BF16, tag="t2")
                        nc.vector.tensor_mul(t2, rot, cs[sinn][:, wcol])
                        nc.vector.tensor_add(dst[m][:, wcol], t1, t2)
                # v for the 4 s-chunks of this block: natural [s, dv] layout.
                # Two s-chunks share one psum bank as a single accumulation
                # group (start on the first chunk's k=0, the second chunk's
                # k=0 overwrites its pending-zero half, stop on its k=7).
                for g in range(2):
                    psv = psp.tile([128, 2, 512], F32, tag="ps", name="psv")
                    for sub in range(4):
                        i = 8 * n + 4 * g + sub
                        scol = slice(128 * i, 128 * (i + 1))
                        half = slice(256 * (sub % 2), 256 * (sub % 2) + 256)
                        for k in range(8):
                            nc.tensor.matmul(
                                psv[:, sub // 2, half],
                                lhsT=xt[k][:, scol],
                                rhs=wv_sb[:, k, :],
                                start=(sub % 2 == 0 and k == 0),
                                stop=(sub % 2 == 1 and k == 7),
                            )
                    for sub in range(4):
                        i = 8 * n + 4 * g + sub
                        half = slice(256 * (sub % 2), 256 * (sub % 2) + 256)
                        # ones columns at 65*h + 64
                        nc.vector.memset(
                            vsb[i].rearrange("p (h c) -> p h c", c=65)[:, :, 64],
                            1.0,
                        )
                        nc.vector.tensor_copy(
                            vsb[i].rearrange("p (h c) -> p h c", c=65)[:, :, 0:64],
                            psv[:, sub // 2, half].rearrange(
                                "p (h c) -> p h c", c=64),
                        )

            # ---- phase 2+3: attention (j-outer, pairs inner), with the
            # normalize + output-projection work emitted as deferred closures
            # woven between later attention tiles, so Scalar/Vector-queue
            # lumps never sit in front of the next block's critical EXP.
            pending = []

            def drain(k=1):
                for _ in range(k):
                    if pending:
                        pending.pop(0)()

            def mk_norm(p, j, h, pvt):
                jcol = slice(512 * j, 512 * (j + 1))
                pvs = smallp.tile([65, 512], F32, tag="pvs", name="pvs")
                nc.scalar.copy(pvs, pvt)
                rd = smallp.tile([1, 512], F32, tag="rd", name="rd")
                nc.vector.reciprocal(rd, pvs[64:65, :])
                rdb = smallp.tile([64, 512], F32, tag="rdb", name="rdb")
                nc.gpsimd.partition_broadcast(rdb, rd)
                nc.vector.tensor_mul(
                    ot[p][64 * h:64 * (h + 1), jcol],
                    pvs[0:64, :],
                    rdb,
                )

            def emit_po(j):
                for m in range(4 * j, 4 * j + 4):
                    mk_po(m)()

            def mk_po(m):
                def go():
                    mcol = slice(128 * m, 128 * (m + 1))
                    posb = bigp.tile([128, D], F32, tag="big", name="posb")
                    for d in range(2):
                        po = pvp.tile([128, 512], F32, tag="pv", name="po")
                        for pp in range(2):
                            nc.tensor.matmul(
                                po,
                                lhsT=ot[pp][:, mcol],
                                rhs=wo_sb[:, pp, 512 * d:512 * (d + 1)],
                                start=(pp == 0),
                                stop=(pp == 1),
                            )
                        nc.scalar.copy(posb[:, 512 * d:512 * (d + 1)], po)
                    nc.sync.dma_start(out=out_d[mcol, :], in_=posb)
                return go

            for j in range(4):
                jcol = slice(512 * j, 512 * (j + 1))
                for p in range(2):
                    pva = pvp.tile([65, 512], F32, tag="pv", name="pva")
                    pvb = pvp.tile([65, 512], F32, tag="pv", name="pvb")
                    pv = (pva, pvb)
                    for i in range(4 * j + 4):
                        r = i - 4 * j
                        loc = max(0, 128 * r)
                        sc = psp.tile([128, 2, 512], F32, tag="ps")
                        icol = slice(128 * i, 128 * (i + 1))
                        for h in range(2):
                            rows = slice(64 * h, 64 * (h + 1))
                            if r < 0:
                                nc.tensor.matmul(
                                    sc[:, h, :],
                                    lhsT=kt[p][rows, icol],
                                    rhs=qt[p][rows, jcol],
                                    start=True,
                                    stop=True,
                                )
                                continue
                            if SIM_SAFE and loc > 0:
                                nc.tensor.matmul(
                                    sc[:, h, 0:loc],
                                    lhsT=tri_sb,
                                    rhs=zeros_sb[:, 0:loc],
                                    start=True,
                                    stop=False,
                                )
                            nc.tensor.matmul(
                                sc[:, h, loc:512],
                                lhsT=kt[p][rows, icol],
                                rhs=qt[p][rows, 512 * j + loc:512 * (j + 1)],
                                start=(loc == 0 or not SIM_SAFE),
                                stop=False,
                                skip_group_check=not SIM_SAFE,
                            )
                            nc.tensor.matmul(
                                sc[:, h, loc:loc + 128],
                                lhsT=tri_sb,
                                rhs=ident_sb,
                                start=False,
                                stop=True,
                            )
                        probs = probsp.tile([128, 2, 512], BF16, tag="probs")
                        nc.scalar.activation(
                            probs[:, :, loc:512], sc[:, :, loc:512], Exp
                        )
                        for h in range(2):
                            hh = 2 * p + h
                            nc.tensor.matmul(
                                pv[h][:, loc:512],
                                lhsT=vsb[i][:, 65 * hh:65 * hh + 65],
                                rhs=probs[:, h, loc:512],
                                start=(i == 0),
                                stop=(i == 4 * j + 3),
                            )
                    mk_norm(p, j, 0, pva)
                    mk_norm(p, j, 1, pvb)
                if j > 0:
                    emit_po(j - 1)
            emit_po(3)

    nc.compile()
    return nc


def _host_inputs(x, Wq, Wk, Wv, Wo, token_positions):
    """Build per-core input maps (all host-side numpy prep)."""
    x = np.asarray(x, dtype=np.float32)
    Wq = np.asarray(Wq, dtype=np.float32)
    Wk = np.asarray(Wk, dtype=np.float32)
    Wv = np.asarray(Wv, dtype=np.float32)
    Wo = np.asarray(Wo, dtype=np.float32)
    pos = np.asarray(token_positions).astype(np.float64)

    # RoPE tables in the permuted-lane layout (16-lane e/o blocks).
    idx = np.arange(0, HD, 2, dtype=np.float64) / HD
    freqs = 1.0 / THETA ** idx                      # [32]
    ang = pos[:, None] * freqs[None, :]             # [S, 32]
    c, s = np.cos(ang).T, np.sin(ang).T             # [32, S]
    c64 = np.concatenate([c[0:16], c[0:16], c[16:32], c[16:32]], 0)
    s64 = np.concatenate([-s[0:16], s[0:16], -s[16:32], s[16:32]], 0)
    cos128 = np.concatenate([c64, c64], 0).astype(np.float32)
    sin128 = np.concatenate([s64, s64], 0).astype(np.float32)
    bf = ml_dtypes.bfloat16
    cosb = cos128.astype(bf)
    sinb = sin128.astype(bf)

    tri = np.where(
        np.arange(128)[:, None] > np.arange(128)[None, :], NEG, 0.0
    ).astype(np.float32)
    tri_lhsT = np.ascontiguousarray(tri.T).astype(bf)
    ident = np.eye(128, dtype=np.float32).astype(bf)

    # per-head row permutation: [e0..e15, o0..o15, e16..e31, o16..o31]
    perm64 = np.concatenate([
        np.arange(0, 32, 2), np.arange(1, 32, 2),
        np.arange(32, 64, 2), np.arange(33, 64, 2),
    ])

    xts = [np.ascontiguousarray(x[b].T) for b in range(B)]

    in_maps = []
    for core in range(NCORES):
        b = core // 4
        heads = [4 * (core % 4) + hh for hh in range(HPC)]
        qk_rows = np.concatenate([g * HD + perm64 for g in heads])
        v_rows = np.concatenate([np.arange(g * HD, (g + 1) * HD) for g in heads])
        in_maps.append({
            "xt": xts[b],
            "wq": np.ascontiguousarray(Wq[qk_rows, :].T) / np.sqrt(HD),
            "wk": np.ascontiguousarray(Wk[qk_rows, :].T),
            "wv": np.ascontiguousarray(Wv[v_rows, :].T),
            "wo": np.ascontiguousarray(Wo[:, v_rows].T).astype(bf),
            "cos": cosb, "sin": sinb,
            "tri": tri_lhsT, "ident": ident,
        })
    return in_maps


def _ensure_ntff_hook():
    """Register the axon NTFF profile hook if the image's antenv lacks it."""
    import sys, types
    try:
        import antenv.axon_hooks  # noqa: F401
        return
    except ImportError:
        pass
    try:
        from trn_agent_boot.trn_boot import _ntff_profile_via_ctypes
        hook = _ntff_profile_via_ctypes("/opt/axon/libaxon_pjrt.so")
    except Exception:
        return
    mod = types.ModuleType("antenv.axon_hooks")
    mod.get_axon_ntff_profile_hook = lambda: hook
    mod.set_axon_ntff_profile_hook = lambda h: None
    sys.modules["antenv.axon_hooks"] = mod


def run(inputs, trace=False):
    """Run the SPMD kernel; returns (full_output, BassKernelResults)."""
    if trace:
        _ensure_ntff_hook()
    if "nc" not in _cached:
        _cached["nc"] = build_nc()
    nc = _cached["nc"]
    in_maps = _host_inputs(
        inputs["x"], inputs["Wq"], inputs["Wk"], inputs["Wv"], inputs["Wo"],
        inputs["token_positions"],
    )
    res = run_bass_kernel_spmd(nc, in_maps, core_ids=list(range(NCORES)),
                               trace=trace)
    out = np.zeros((B, S, D), dtype=np.float32)
    for core in range(NCORES):
        out[core // 4] += res.results[core]["out"]
    return out, res


def kernel(**inputs) -> np.ndarray:
    out, _ = run(inputs, trace=False)
    return out

